# revision 4
# baseline (speedup 1.0000x reference)
"""YOLOv1 loss (nn_LossModul_16277926052544) on 8 TRN2 NeuronCores, v6.

v6 = v4 with predicts/targets interleaved into ONE dram tensor per core:
3 loads per tile instead of 6 (each DMA_DIRECT2D issue costs ~650ns on the
sync sequencer and they serialize ahead of the data), and the two diff
adds read both halves from one tile.
"""
import sys

for _p in ("/opt/trn_rl_repo",):
    if _p not in sys.path:
        sys.path.insert(0, _p)

import numpy as np
import ml_dtypes
from contextlib import ExitStack

import concourse.bass as bass  # noqa: F401  (registers engines)
from concourse import bacc, mybir
from concourse import bass_utils
import concourse.tile as tile

N_CORES = 8
BATCH = 8192
S = 7
P = 128
CELLS_PER_CORE = (BATCH // N_CORES) * S * S   # 50176
T_TILES = 2
F = CELLS_PER_CORE // P // T_TILES            # 196
R = 1.0 / S
EPS = 1e-6
SQ5 = float(np.sqrt(5.0))
SQH = float(np.sqrt(0.5))
UC = 71                                       # merged channels (see _pack)

f32 = mybir.dt.float32
bf16 = mybir.dt.bfloat16
u32 = mybir.dt.uint32
Alu = mybir.AluOpType
Act = mybir.ActivationFunctionType

_CACHE = {}


def _build_body(tc_, ctx, u_d, out_ap):
    nc = tc_.nc
    inpool = ctx.enter_context(tc_.tile_pool(name="in", bufs=2))
    wkp = ctx.enter_context(tc_.tile_pool(name="wk", bufs=2))
    stp = ctx.enter_context(tc_.tile_pool(name="st", bufs=1))
    stats = stp.tile([P, 2 * T_TILES], f32)
    eps5c = stp.tile([P, 1], f32)               # bias const for Sqrt
    nc.gpsimd.memset(eps5c[:], 5.0 * EPS)
    dummy = stp.tile([P, 1], f32)
    # act-table priming: first activation picks the set holding
    # sqrt+abs+square+copy so no reload happens mid-kernel
    nc.scalar.activation(dummy[:], eps5c[:], Act.Sqrt)

    for t in range(T_TILES):
        # dxyt: 0:4 sqrt5*pxy | 4:8 -sqrt5*txy
        dxyt = inpool.tile([P, 8, F], bf16, tag="dxyt")
        # pt: 0:4 pwh | 4:8 2pwh | 8:11 b1 | 11:14 b2
        #     14:18 twh | 18:22 2twh | 22 tc
        pt = inpool.tile([P, 23, F], bf16, tag="pt")
        # clst: 0:20 pcls | 20:40 -tcls
        clst = inpool.tile([P, 40, F], bf16, tag="clst")
        # WK zones: 0:4 dxy5 | 4:24 clsd | 24:26 tm->vnoobj | 26:33 SEL
        # SEL = [sx, sy, sw, sh, sc, sI, sD]
        wk = wkp.tile([P, 33, F], bf16, tag="wk")

        nc.sync.dma_start(dxyt[:], u_d[:, t, 0:8])
        nc.sync.dma_start(pt[:], u_d[:, t, 8:31])
        nc.sync.dma_start(clst[:], u_d[:, t, 31:71])

        Sg = wkp.tile([P, 8, F], bf16, tag="S")      # geometry scratch
        absd = wkp.tile([P, 4, F], bf16, tag="absd")
        idt = wkp.tile([P, 2, 2, F], bf16, tag="idt")  # [I|D, box]
        g = wkp.tile([P, 2, F], f32, tag="g")
        respb = wkp.tile([P, 1, F], u32, tag="respb")
        mob = wkp.tile([P, 1, F], bf16, tag="mob")
        mnh = wkp.tile([P, 1, F], bf16, tag="mnh")
        s2t = wkp.tile([P, 2, F], bf16, tag="s2t")
        at4 = wkp.tile([P, 1, F], bf16, tag="at4")
        dself = wkp.tile([P, 1, F], f32, tag="dself")
        rcpt = wkp.tile([P, 1, F], f32, tag="rcpt")
        ioub = wkp.tile([P, 1, F], bf16, tag="ioub")

        def bc(ap_p1f, k):
            return ap_p1f.broadcast_to([P, k, F])

        # ---------- diffs (host negated the target halves) ----------
        nc.vector.tensor_add(wk[:, 0:4], dxyt[:, 0:4], dxyt[:, 4:8])
        nc.vector.tensor_add(wk[:, 4:24], clst[:, 0:20], clst[:, 20:40])

        # ---------- geometry (x4-scaled IoU, all 2x TTs) ----------
        nc.scalar.activation(absd[:], wk[:, 0:4], Act.Abs,
                             scale=2.0 * R / SQ5)        # 2R|dxy|
        nc.vector.tensor_add(Sg[:, 0:4], pt[:, 0:4], pt[:, 14:18])   # pw+tw
        nc.vector.tensor_sub(Sg[:, 4:8], Sg[:, 0:4], absd[:])        # m2
        nc.vector.tensor_tensor(Sg[:, 0:4], pt[:, 4:8], pt[:, 18:22],
                                op=Alu.min)                          # mn2
        nc.vector.tensor_scalar_max(Sg[:, 4:8], Sg[:, 4:8], 0.0)
        nc.vector.tensor_tensor(Sg[:, 4:8], Sg[:, 4:8], Sg[:, 0:4],
                                op=Alu.min)                          # ln2
        nc.vector.tensor_mul(idt[:, 0], Sg[:, 4:8:2], Sg[:, 5:8:2])  # I4
        nc.gpsimd.tensor_mul(at4[:], pt[:, 18:19], pt[:, 19:20])     # At4
        nc.vector.tensor_mul(idt[:, 1], pt[:, 4:8:2], pt[:, 5:8:2])  # A4
        nc.vector.tensor_sub(idt[:, 1], idt[:, 1], idt[:, 0])        # A4-I4
        nc.vector.tensor_add(idt[:, 1], idt[:, 1], bc(at4[:], 2))    # D4
        nc.vector.tensor_mul(g[:, 0:1], idt[:, 0, 0:1], idt[:, 1, 1:2])
        nc.vector.tensor_mul(g[:, 1:2], idt[:, 0, 1:2], idt[:, 1, 0:1])
        nc.vector.tensor_tensor(respb[:], g[:, 0:1], g[:, 1:2], op=Alu.is_gt)

        # ---------- masks ----------
        nc.vector.tensor_scalar(mob[:], pt[:, 22:23], 0.0, 1.0,
                                op0=Alu.is_gt, op1=Alu.mult)
        nc.vector.tensor_scalar(mnh[:], pt[:, 22:23], 0.0, SQH,
                                op0=Alu.is_le, op1=Alu.mult)

        # ---------- selection (box2 copied, box1 predicated over) ----------
        nc.scalar.copy(wk[:, 24:26], pt[:, 14:16])       # tm source (tw,th)
        nc.vector.tensor_copy(wk[:, 26:28], wk[:, 2:4])  # d2xy
        nc.scalar.copy(wk[:, 28:31], pt[:, 11:14])       # w2,h2,c2
        nc.vector.tensor_copy(wk[:, 31:33], idt[:, :, 1])  # I2,D2
        nc.vector.copy_predicated(wk[:, 26:28], bc(respb[:], 2), wk[:, 0:2])
        nc.vector.copy_predicated(wk[:, 28:31], bc(respb[:], 3), pt[:, 8:11])
        nc.vector.copy_predicated(wk[:, 31:33], bc(respb[:], 2), idt[:, :, 0])

        # ---------- masks applied: cls zone and box zone ----------
        nc.vector.tensor_mul(wk[:, 4:24], wk[:, 4:24], bc(mob[:], 20))
        nc.vector.tensor_mul(wk[:, 24:32], wk[:, 24:32], bc(mob[:], 8))

        # ---------- cls Square+accumulate (early, independent) ----------
        nc.scalar.activation(wk[:, 4:24], wk[:, 4:24], Act.Square,
                             accum_out=stats[:, 2 * t:2 * t + 1])

        # ---------- iou / conf ----------
        nc.vector.tensor_copy(dself[:], wk[:, 32:33])    # D4sel -> f32
        nc.vector.reciprocal_approx_fast(rcpt[:], dself[:])
        nc.vector.tensor_mul(ioub[:], wk[:, 31:32], rcpt[:])  # mo*iou
        nc.vector.scalar_tensor_tensor(                  # csel = mo*pc - mo*iou
            wk[:, 30:31], ioub[:], -1.0, wk[:, 30:31],
            op0=Alu.mult, op1=Alu.add)

        # ---------- wh sqrt diff (x5 inside sqrt) ----------
        nc.scalar.activation(wk[:, 28:30], wk[:, 28:30], Act.Sqrt,
                             scale=5.0, bias=eps5c[:])
        nc.scalar.activation(s2t[:], wk[:, 24:26], Act.Sqrt,
                             scale=5.0, bias=eps5c[:])
        nc.vector.tensor_sub(wk[:, 28:30], wk[:, 28:30], s2t[:])

        # ---------- noobj conf (overwrites tm slots) ----------
        nc.vector.tensor_mul(wk[:, 24:26], pt[:, 10:14:3], bc(mnh[:], 2))

        # ---------- box-zone Square+accumulate ----------
        nc.scalar.activation(wk[:, 24:31], wk[:, 24:31], Act.Square,
                             accum_out=stats[:, 2 * t + 1:2 * t + 2])

    psum = ctx.enter_context(tc_.psum_pool(name="ps", bufs=1))
    acc = psum.tile([1, 2 * T_TILES], f32)
    ones = stp.tile([P, 1], f32)
    nc.gpsimd.memset(ones[:], 1.0)
    nc.tensor.matmul(acc[:], ones[:], stats[:], start=True, stop=True)
    osb = stp.tile([1, 2 * T_TILES], f32)
    nc.scalar.copy(osb[:], acc[:])
    nc.sync.dma_start(out_ap, osb[:])


def _build():
    if "nc" in _CACHE:
        return _CACHE["nc"]
    nc = bacc.Bacc("TRN2", target_bir_lowering=False, debug=False)
    u = nc.dram_tensor("u", [P, T_TILES, UC, F], bf16, kind="ExternalInput")
    out = nc.dram_tensor("out", [1, 2 * T_TILES], f32, kind="ExternalOutput")
    with tile.TileContext(nc) as tc_, ExitStack() as ctx:
        _build_body(tc_, ctx, u.ap(), out.ap())
    nc.compile()
    _CACHE["nc"] = nc
    return nc


def _pack(predicts, targets):
    """Per-core host prep: shard, merge both tensors channel-wise with
    reorder/dup/pre-scale/negate, cast bf16, to channel-planar [P,T,C,F]."""
    p = np.ascontiguousarray(predicts, dtype=np.float32)
    t = np.ascontiguousarray(targets, dtype=np.float32)
    n = BATCH // N_CORES
    us = []
    for i in range(N_CORES):
        ps = p[i * n:(i + 1) * n].reshape(-1, 30)   # [50176, 30]
        ts = t[i * n:(i + 1) * n].reshape(-1, 30)
        u = np.empty((ps.shape[0], UC), np.float32)
        u[:, 0:4] = SQ5 * ps[:, [0, 1, 5, 6]]       # sqrt5*xy both boxes
        u[:, 4:8] = -SQ5 * ts[:, [0, 1, 0, 1]]      # negated target xy
        u[:, 8:12] = ps[:, [2, 3, 7, 8]]            # w1,h1,w2,h2
        u[:, 12:16] = 2.0 * ps[:, [2, 3, 7, 8]]     # 2*wh both boxes
        u[:, 16:19] = ps[:, [2, 3, 4]]              # box1 w,h,c
        u[:, 19:22] = ps[:, [7, 8, 9]]              # box2 w,h,c
        u[:, 22:26] = ts[:, [2, 3, 2, 3]]           # twh dup
        u[:, 26:30] = 2.0 * ts[:, [2, 3, 2, 3]]     # 2*twh dup
        u[:, 30] = ts[:, 4]                         # tc
        u[:, 31:51] = ps[:, 10:30]                  # pcls
        u[:, 51:71] = -ts[:, 10:30]                 # negated tcls
        u = u.astype(ml_dtypes.bfloat16).reshape(P, T_TILES, F, UC)
        us.append(np.ascontiguousarray(u.transpose(0, 1, 3, 2)))
    return us


def run(predicts, targets, trace=False, **trace_kwargs):
    nc = _build()
    us = _pack(predicts, targets)
    in_maps = [{"u": us[i]} for i in range(N_CORES)]
    res = bass_utils.run_bass_kernel_spmd(
        nc, in_maps, core_ids=list(range(N_CORES)), trace=trace,
        **trace_kwargs)
    partial = np.zeros((), dtype=np.float64)
    for r in res.results:
        partial += np.asarray(r["out"], dtype=np.float64).sum()
    return np.float32(partial), res


def kernel(predicts, targets):
    out, _ = run(predicts, targets, trace=False)
    return out
